# revision 1
# baseline (speedup 1.0000x reference)
"""Fused multi-head attention on 8 Trainium2 NeuronCores.

Problem: x[2,2048,1024] -> qkv proj (16 heads, hd=64) -> softmax attention
-> out proj.  Sharding: tensor parallel over heads, 2 heads per core.
Each core computes q/k/v for its 2 heads, full attention for its
4 (batch, head) pairs, and the partial out-projection contribution of its
128 head-dims.  Host sums the 8 partial outputs and adds out_b.

Layouts on device (per core):
  xT    [1024, 4096]  bf16   hidden on partitions, tokens free (b-major)
  qkvT  [128, 4096]   bf16   per group; head A dims on partitions 0-63, B on 64-127
  scores^T in PSUM: [k-tile 128, q 512] per head, heads packed side by side
  p = exp(scores/8) in SBUF bf16 (no max subtraction: |scores/8| < ~3)
  PV: lhsT = v_aug [k-tile 128, 65] slices of a combined per-batch tile
      holding [pad | onesA | v(A,B) transposed | onesB]; the ones column
      produces the softmax colsum as an extra row of o (row 0 for head A,
      row 64 for head B).
  out-proj per head: lhsT = o_aug^T [65, t-tile 128], rhs = woT_aug [65, 1025]
    (the unit row of rhs is zero in cols 0-1023 and 1 in col 1024, so col
     1024 of the output is the per-token colsum, transposed to partitions)
  y = recipA * y_A + recipB * y_B on DVE (per-partition scalars), f32 out.

The attention loop is software-pipelined: PV lags one k-tile behind
scores/exp, and the previous q-chunk's out-projection epilogue is
interleaved into the current chunk's k-loop so the PE never waits on the
scalar engine's exp.
"""

import sys
import types
import numpy as np
import ml_dtypes

import concourse.bass as bass
import concourse.tile as tile
from concourse import bacc, mybir

BF16 = mybir.dt.bfloat16
F32 = mybir.dt.float32
BF16_NP = ml_dtypes.bfloat16

B, S, H, NH, HD = 2, 2048, 1024, 16, 64
T = B * S               # 4096 tokens, b-major
NCORES = 8
HPC = NH // NCORES      # heads per core = 2
DPC = HPC * HD          # head dims per core = 128
KT = 128                # keys per k-tile
NKT = S // KT           # 16
QC = 512                # query chunk
NQC = S // QC           # 4
HKT = H // 128          # hidden k-tiles = 8
VS = 160                # v_aug stride per k-tile (pad15|onesA|v 128|onesB|pad15)
VOFF = 16               # col offset of the transposed v block within a stride
EXPSCALE = 1.0 / np.sqrt(HD)

_CACHED = {}


def _build_nc():
    nc = bacc.Bacc(None, target_bir_lowering=False, debug=False)
    xT = nc.dram_tensor("xT", [H, T], BF16, kind="ExternalInput").ap()
    wqkvT = nc.dram_tensor("wqkvT", [H, 3 * DPC], BF16, kind="ExternalInput").ap()
    bqkv = nc.dram_tensor("bqkv", [DPC, 3], F32, kind="ExternalInput").ap()
    woTa = nc.dram_tensor("woTa", [HPC, HD + 1, H + 1], BF16, kind="ExternalInput").ap()
    vbias = nc.dram_tensor("vbias", [128, DPC], F32, kind="ExternalInput").ap()
    out = nc.dram_tensor("out", [T, H], F32, kind="ExternalOutput").ap()

    EXP = mybir.ActivationFunctionType.Exp
    MULT = mybir.AluOpType.mult
    ADD = mybir.AluOpType.add

    with tile.TileContext(nc) as tc:
        with (
            tc.tile_pool(name="const", bufs=1) as constp,
            tc.tile_pool(name="xw", bufs=1) as xwp,
            tc.tile_pool(name="qkv", bufs=1) as qkvp,
            tc.tile_pool(name="vaug", bufs=1) as vaugp,
            tc.tile_pool(name="oT", bufs=4) as oTp,
            tc.tile_pool(name="p", bufs=3) as pp,
            tc.tile_pool(name="ysb", bufs=3) as ysbp,
            tc.tile_pool(name="small", bufs=4) as smallp,
            tc.tile_pool(name="ps", bufs=2, space="PSUM") as psp,
        ):
            # ---- constants / weights in ----
            bias_sb = constp.tile([DPC, 3], F32, tag="bias")
            nc.sync.dma_start(bias_sb[:], bqkv[:])
            vbias_sb = constp.tile([128, DPC], F32, tag="vbias")
            nc.sync.dma_start(vbias_sb[:], vbias[:])
            wo_sb = [
                constp.tile([HD + 1, H + 1], BF16, name=f"wo{h}", tag=f"wo{h}")
                for h in range(HPC)
            ]
            for h in range(HPC):
                nc.sync.dma_start(wo_sb[h][:], woTa[h])

            # ---- x and qkv weights in ----
            xT_sb = [xwp.tile([128, T], BF16, name=f"xsb{k}", tag=f"x{k}")
                     for k in range(HKT)]
            wq_sb = [xwp.tile([128, 3 * DPC], BF16, name=f"wsb{k}", tag=f"w{k}")
                     for k in range(HKT)]
            for k in range(HKT):
                nc.sync.dma_start(wq_sb[k][:], wqkvT[k * 128:(k + 1) * 128, :])
            for k in range(HKT):
                eng = nc.sync if k % 2 == 0 else nc.scalar
                eng.dma_start(xT_sb[k][:], xT[k * 128:(k + 1) * 128, :])

            # vaug tiles (memset to 1.0 early so the ones columns are ready)
            vaug = {}
            for b in range(B):
                va = vaugp.tile([128, NKT * VS], BF16, name=f"va{b}", tag=f"va{b}")
                nc.vector.memset(va[:], 1.0)
                vaug[b] = va

            # ---- qkv projection ----
            # v is computed directly in token-major layout (x^T stationary),
            # written straight into the v_aug tiles; q/k are feature-major,
            # weight-stationary, split by batch so batch-0 attention starts
            # early.
            qkvT_sb = {
                fg: qkvp.tile([128, T], BF16, name=f"qkvsb{fg}", tag=f"qkv{fg}")
                for fg in range(2)
            }
            ADDOP = mybir.AluOpType.add

            def v_tile_step(b, kt):
                def run():
                    v_ps = psp.tile([128, DPC], F32, name=f"vps{b}{kt}", tag="y")
                    for k in range(HKT):
                        nc.tensor.matmul(
                            v_ps[:],
                            lhsT=xT_sb[k][:, b * S + kt * KT:b * S + (kt + 1) * KT],
                            rhs=wq_sb[k][:, 2 * DPC:3 * DPC],
                            start=(k == 0), stop=(k == HKT - 1),
                        )
                    nc.vector.tensor_add(
                        vaug[b][:, kt * VS + VOFF:kt * VS + VOFF + DPC],
                        v_ps[:], vbias_sb[:],
                    )
                return run

            def qk_group(fg, half):
                tiles = [
                    psp.tile([128, 1024], F32, name=f"qp{fg}{half}a", tag="s"),
                    psp.tile([128, 512], F32, name=f"qp{fg}{half}c", tag="o"),
                    psp.tile([128, 512], F32, name=f"qp{fg}{half}d", tag="y"),
                ]

                def tc_slice(t):
                    if t < 2:
                        return tiles[0][:, t * 512:(t + 1) * 512]
                    return tiles[t - 1][:]

                for k in range(HKT):
                    for t in range(4):
                        nc.tensor.matmul(
                            tc_slice(t),
                            lhsT=wq_sb[k][:, fg * DPC:(fg + 1) * DPC],
                            rhs=xT_sb[k][:, half * S + t * 512:half * S + (t + 1) * 512],
                            start=(k == 0),
                            stop=(k == HKT - 1),
                        )
                for t in range(4):
                    nc.vector.tensor_scalar_add(
                        qkvT_sb[fg][:, half * S + t * 512:half * S + (t + 1) * 512],
                        tc_slice(t), bias_sb[:, fg:fg + 1],
                    )

            qk_group(0, 0)
            qk_group(1, 0)

            qT_sb, kT_sb = qkvT_sb[0], qkvT_sb[1]

            def va_lhsT(b, h, kt):
                # head 0: [onesA | vA] cols 15..79 ; head 1: [vB | onesB] 80..144
                c0 = kt * VS + (15 if h == 0 else 80)
                return vaug[b][:, c0:c0 + HD + 1]

            def qk_chunk_step(fg, half, t):
                def run():
                    tg = ("y", "y")[t % 2]
                    ps = psp.tile([128, 512], F32, name=f"qkc{fg}{half}{t}", tag=tg)
                    for k in range(HKT):
                        nc.tensor.matmul(
                            ps[:],
                            lhsT=wq_sb[k][:, fg * DPC:(fg + 1) * DPC],
                            rhs=xT_sb[k][:, half * S + t * 512:half * S + (t + 1) * 512],
                            start=(k == 0),
                            stop=(k == HKT - 1),
                        )
                    nc.vector.tensor_scalar_add(
                        qkvT_sb[fg][:, half * S + t * 512:half * S + (t + 1) * 512],
                        ps[:], bias_sb[:, fg:fg + 1],
                    )
                return run

            # ---- attention + pipelined out-projection epilogue ----
            # pending: filler steps (v tiles, b1 projections, epilogues) run
            # one-or-two per k-tile inside the attention loops.
            pending = [v_tile_step(0, kt) for kt in range(NKT)]
            extra = [v_tile_step(1, kt) for kt in range(NKT)]
            extra += [qk_chunk_step(fg, 1, t) for fg in (0, 1) for t in range(4)]

            def make_epilogue(b, qc, oT):
                q0 = b * S + qc * QC
                state = {}

                def ct_step():
                    ct = [psp.tile([128, 4], F32, name=f"ct{b}{qc}{h}", tag="y")
                          for h in range(HPC)]
                    rT = smallp.tile([128, 2 * 4], F32, name=f"rT{b}{qc}", tag="rT")
                    for h in range(HPC):
                        for tt in range(4):
                            nc.tensor.matmul(
                                ct[h][:, tt:tt + 1],
                                lhsT=oT[h][:, tt * KT:(tt + 1) * KT],
                                rhs=wo_sb[h][:, H:H + 1],
                                start=True, stop=True,
                            )
                        nc.vector.reciprocal(rT[:, h * 4:h * 4 + 4], ct[h][:])
                    state["rT"] = rT

                steps = [ct_step]

                def y_step(tt, ec):
                    def run():
                        rT = state["rT"]
                        y_ps = [
                            psp.tile([128, 512], F32, name=f"y{b}{qc}{tt}{ec}{h}",
                                     tag="y")
                            for h in range(HPC)
                        ]
                        for h in range(HPC):
                            nc.tensor.matmul(
                                y_ps[h][:],
                                lhsT=oT[h][:, tt * KT:(tt + 1) * KT],
                                rhs=wo_sb[h][:, ec * 512:(ec + 1) * 512],
                                start=True, stop=True,
                            )
                        y_sb = ysbp.tile([128, 512], F32, name=f"ys{b}{qc}{tt}{ec}",
                                         tag="ysb")
                        nc.vector.tensor_scalar_mul(
                            y_sb[:], y_ps[0][:], rT[:, 0 * 4 + tt:0 * 4 + tt + 1])
                        nc.vector.scalar_tensor_tensor(
                            y_sb[:], y_ps[1][:], rT[:, 1 * 4 + tt:1 * 4 + tt + 1],
                            y_sb[:], op0=MULT, op1=ADD,
                        )
                        nc.sync.dma_start(
                            out[q0 + tt * KT:q0 + (tt + 1) * KT,
                                ec * 512:(ec + 1) * 512],
                            y_sb[:],
                        )
                    return run

                for tt in range(4):
                    for ec in range(2):
                        steps.append(y_step(tt, ec))
                return steps

            for b in range(B):
                for qc in range(NQC):
                    q0 = b * S + qc * QC
                    o_ps = [psp.tile([HD + 1, QC], F32, name=f"o{b}{qc}{h}", tag="o")
                            for h in range(HPC)]
                    p_tiles = []
                    for kt in range(NKT):
                        s_ps = psp.tile([128, HPC * QC], F32, tag="s")
                        for h in range(HPC):
                            nc.tensor.matmul(
                                s_ps[:, h * QC:(h + 1) * QC],
                                lhsT=kT_sb[h * HD:(h + 1) * HD,
                                           b * S + kt * KT:b * S + (kt + 1) * KT],
                                rhs=qT_sb[h * HD:(h + 1) * HD, q0:q0 + QC],
                                start=True, stop=True,
                            )
                        p_sb = pp.tile([128, HPC * QC], BF16, tag="p")
                        nc.scalar.activation(p_sb[:], s_ps[:], EXP,
                                             scale=float(EXPSCALE))
                        p_tiles.append(p_sb)
                        if pending:
                            pending.pop(0)()
                        if pending and len(pending) > NKT - 1 - kt:
                            pending.pop(0)()
                        if kt >= 2:
                            for h in range(HPC):
                                nc.tensor.matmul(
                                    o_ps[h][:],
                                    lhsT=va_lhsT(b, h, kt - 2),
                                    rhs=p_tiles[kt - 2][:, h * QC:(h + 1) * QC],
                                    start=(kt - 2 == 0), stop=False,
                                )
                    for lag in (NKT - 2, NKT - 1):
                        for h in range(HPC):
                            nc.tensor.matmul(
                                o_ps[h][:],
                                lhsT=va_lhsT(b, h, lag),
                                rhs=p_tiles[lag][:, h * QC:(h + 1) * QC],
                                start=False, stop=(lag == NKT - 1),
                            )
                    oT = [oTp.tile([HD + 1, QC], BF16, name=f"oT{b}{qc}{h}", tag="oT")
                          for h in range(HPC)]
                    for h in range(HPC):
                        nc.vector.tensor_copy(oT[h][:], o_ps[h][:])
                    while pending:
                        pending.pop(0)()
                    epi = make_epilogue(b, qc, oT)
                    take = min(len(extra), NKT - len(epi))
                    pending = extra[:take] + epi
                    del extra[:take]
            while pending:
                pending.pop(0)()
    nc.compile()
    return nc


def _get_nc():
    if "nc" not in _CACHED:
        _CACHED["nc"] = _build_nc()
    return _CACHED["nc"]


def _host_prep(x, qkv_w, qkv_b, out_w):
    x = np.asarray(x, dtype=np.float32)
    qkv_w = np.asarray(qkv_w, dtype=np.float32)
    qkv_b = np.asarray(qkv_b, dtype=np.float32)
    out_w = np.asarray(out_w, dtype=np.float32)

    xT = np.ascontiguousarray(x.reshape(T, H).T).astype(BF16_NP)
    in_maps = []
    for c in range(NCORES):
        wq = qkv_w[128 * c:128 * c + 128]
        wk = qkv_w[H + 128 * c:H + 128 * c + 128]
        wv = qkv_w[2 * H + 128 * c:2 * H + 128 * c + 128]
        wqkvT = np.ascontiguousarray(np.concatenate([wq, wk, wv], 0).T).astype(BF16_NP)
        bq = np.stack(
            [qkv_b[fg * H + 128 * c:fg * H + 128 * c + 128] for fg in range(3)],
            axis=1,
        ).astype(np.float32)
        woTa = np.zeros((HPC, HD + 1, H + 1), np.float32)
        for h in range(HPC):
            g = HPC * c + h
            w = out_w[:, g * HD:(g + 1) * HD].T  # [64, 1024]
            if h == 0:
                # head A: colsum is row 0 of o_aug (ones col precedes v)
                woTa[h, 1:HD + 1, 0:H] = w
                woTa[h, 0, H] = 1.0
            else:
                # head B: colsum is row 64 (ones col follows v)
                woTa[h, 0:HD, 0:H] = w
                woTa[h, HD, H] = 1.0
        vb = np.broadcast_to(
            qkv_b[2 * H + 128 * c:2 * H + 128 * c + 128][None, :], (128, DPC)
        ).astype(np.float32)
        in_maps.append({
            "xT": xT,
            "wqkvT": wqkvT,
            "bqkv": np.ascontiguousarray(bq),
            "woTa": woTa.astype(BF16_NP),
            "vbias": np.ascontiguousarray(vb),
        })
    return in_maps


def _run(in_maps, trace=False):
    # The image's antenv lacks axon_hooks; register the NTFF profile hook so
    # run_bass_kernel_spmd(trace=True) can report exec_time_ns.
    if trace and "antenv.axon_hooks" not in sys.modules:
        try:
            import trn_agent_boot.trn_boot as _tb
            _hook = _tb._ntff_profile_via_ctypes("/opt/axon/libaxon_pjrt.so")
            _m = types.ModuleType("antenv.axon_hooks")
            _m.get_axon_ntff_profile_hook = lambda: _hook
            sys.modules["antenv.axon_hooks"] = _m
        except Exception:
            trace = False
    from concourse.bass_utils import run_bass_kernel_spmd

    nc = _get_nc()
    res = run_bass_kernel_spmd(nc, in_maps, core_ids=list(range(NCORES)), trace=trace)
    return res


def kernel(x, qkv_w, qkv_b, out_w, out_b):
    in_maps = _host_prep(x, qkv_w, qkv_b, out_w)
    res = _run(in_maps, trace=False)
    total = np.zeros((T, H), np.float32)
    for c in range(NCORES):
        total += res.results[c]["out"]
    total += np.asarray(out_b, dtype=np.float32)[None, :]
    return total.reshape(B, S, H)



# revision 11
# speedup vs baseline: 1.1550x; 1.1550x over previous
"""Fused multi-head attention on 8 Trainium2 NeuronCores.

Problem: x[2,2048,1024] -> qkv proj (16 heads, hd=64) -> softmax attention
-> out proj.  Sharding: tensor parallel over heads, 2 heads per core.
Each core computes q/k/v for its 2 heads, full attention for its
4 (batch, head) pairs, and the partial out-projection contribution of its
128 head-dims.  Host sums the 8 partial outputs and adds out_b.

Layouts on device (per core):
  xT    [1024, 4096]  bf16   hidden on partitions, tokens free (b-major)
  qkvT  [128, 4096]   bf16   per group; head A dims on partitions 0-63, B on 64-127
  scores^T in PSUM: [k-tile 128, q 512] per head, heads packed side by side
  p = exp(scores/8) in SBUF bf16 (no max subtraction: |scores/8| < ~3)
  PV: lhsT = v_aug [k-tile 128, 65] slices of a combined per-batch tile
      holding [pad | onesA | v(A,B) transposed | onesB]; the ones column
      produces the softmax colsum as an extra row of o (row 0 for head A,
      row 64 for head B).
  out-proj per head: lhsT = o_aug^T [65, t-tile 128], rhs = woT_aug [65, 1025]
    (the unit row of rhs is zero in cols 0-1023 and 1 in col 1024, so col
     1024 of the output is the per-token colsum, transposed to partitions)
  y = recipA * y_A + recipB * y_B on DVE (per-partition scalars), f32 out.

The attention loop is software-pipelined: PV lags one k-tile behind
scores/exp, and the previous q-chunk's out-projection epilogue is
interleaved into the current chunk's k-loop so the PE never waits on the
scalar engine's exp.
"""

import sys
import types
import numpy as np
import ml_dtypes

import concourse.bass as bass
import concourse.tile as tile
from concourse import bacc, mybir

BF16 = mybir.dt.bfloat16
F32 = mybir.dt.float32
FP8 = mybir.dt.float8e4
BF16_NP = ml_dtypes.bfloat16

B, S, H, NH, HD = 2, 2048, 1024, 16, 64
T = B * S               # 4096 tokens, b-major
NCORES = 8
HPC = NH // NCORES      # heads per core = 2
DPC = HPC * HD          # head dims per core = 128
KT = 128                # keys per k-tile
NKT = S // KT           # 16
QC = 512                # query chunk
NQC = S // QC           # 4
HKT = H // 128          # hidden k-tiles = 8
VS = 160                # v_aug stride per k-tile (pad15|onesA|v 128|onesB|pad15)
VOFF = 16               # col offset of the transposed v block within a stride
EXPSCALE = 1.0 / np.sqrt(HD)

_CACHED = {}


def _build_nc():
    nc = bacc.Bacc(None, target_bir_lowering=False, debug=False)
    xT = nc.dram_tensor("xT", [H, T], BF16, kind="ExternalInput").ap()
    wqkvT = nc.dram_tensor("wqkvT", [H, 3 * DPC], BF16, kind="ExternalInput").ap()
    bqkv = nc.dram_tensor("bqkv", [DPC, 3], F32, kind="ExternalInput").ap()
    woTa = nc.dram_tensor("woTa", [HPC, HD + 1, H + 1], BF16, kind="ExternalInput").ap()
    vbias = nc.dram_tensor("vbias", [128, DPC], F32, kind="ExternalInput").ap()
    out = nc.dram_tensor("out", [T, H], BF16, kind="ExternalOutput").ap()

    EXP = mybir.ActivationFunctionType.Exp
    MULT = mybir.AluOpType.mult
    ADD = mybir.AluOpType.add

    with tile.TileContext(nc) as tc:
        with (
            tc.tile_pool(name="const", bufs=1) as constp,
            tc.tile_pool(name="xw", bufs=1) as xwp,
            tc.tile_pool(name="qkv", bufs=1) as qkvp,
            tc.tile_pool(name="vaug", bufs=1) as vaugp,
            tc.tile_pool(name="oT", bufs=4) as oTp,
            tc.tile_pool(name="p", bufs=3) as pp,
            tc.tile_pool(name="ysb", bufs=3) as ysbp,
            tc.tile_pool(name="small", bufs=4) as smallp,
            tc.tile_pool(name="ps", bufs=2, space="PSUM") as psp,
        ):
            # ---- x and qkv weights in ----
            # Priority order: qkv weights + batch-0 x first (so the batch-0
            # q/k projection can start within a few us), then constants, then
            # batch-1 x.  Two DMA queues (sync / scalar) run in parallel;
            # per-queue program order is the issue order below.
            xT_sb = [xwp.tile([128, T], BF16, name=f"xsb{k}", tag=f"x{k}")
                     for k in range(HKT)]
            wq_sb = [xwp.tile([128, 3 * DPC], BF16, name=f"wsb{k}", tag=f"w{k}")
                     for k in range(HKT)]
            # weights: odd k on sync, even k on scalar (so each queue's first
            # transfers are small and the matching xT tile follows promptly)
            for k in range(HKT):
                eng = nc.scalar if k % 2 == 0 else nc.sync
                eng.dma_start(wq_sb[k][:], wqkvT[k * 128:(k + 1) * 128, :])
            # batch-0 halves of x, alternating queues, in k order
            for k in range(HKT):
                eng = nc.sync if k % 2 == 0 else nc.scalar
                eng.dma_start(xT_sb[k][:, 0:S], xT[k * 128:(k + 1) * 128, 0:S])

            # constants (needed later than x/w: bias after first qk matmuls,
            # vbias for v steps, wo for epilogues)
            bias_sb = constp.tile([DPC, 3], F32, tag="bias")
            nc.sync.dma_start(bias_sb[:], bqkv[:])
            vbias_sb = constp.tile([128, DPC], F32, tag="vbias")
            nc.scalar.dma_start(vbias_sb[:], vbias[:])
            wo_sb = [
                constp.tile([HD + 1, H + 1], BF16, name=f"wo{h}", tag=f"wo{h}")
                for h in range(HPC)
            ]
            for h in range(HPC):
                nc.sync.dma_start(wo_sb[h][:], woTa[h])

            # batch-1 halves of x
            for k in range(HKT):
                eng = nc.scalar if k % 2 == 0 else nc.sync
                eng.dma_start(xT_sb[k][:, S:T], xT[k * 128:(k + 1) * 128, S:T])

            # vaug tiles in fp8, paired by k-tile for DoubleRow PV:
            # [128, pair, ko, VS] with ko the pair-member axis.
            # (memset to 1.0 early so the ones columns are ready)
            vaug = {}
            for b in range(B):
                va = vaugp.tile([128, NKT // 2, 2, VS], FP8, name=f"va{b}",
                                tag=f"va{b}")
                nc.vector.memset(va[:], 1.0)
                vaug[b] = va

            # ---- qkv projection ----
            # v is computed directly in token-major layout (x^T stationary),
            # written straight into the v_aug tiles; q/k are feature-major,
            # weight-stationary, split by batch so batch-0 attention starts
            # early.
            qkvT_sb = {
                fg: qkvp.tile([128, T], BF16, name=f"qkvsb{fg}", tag=f"qkv{fg}")
                for fg in range(2)
            }
            ADDOP = mybir.AluOpType.add

            def v_tile_step(b, kt):
                def run():
                    v_ps = psp.tile([128, DPC], F32, name=f"vps{b}{kt}", tag="y")
                    for k in range(HKT):
                        nc.tensor.matmul(
                            v_ps[:],
                            lhsT=xT_sb[k][:, b * S + kt * KT:b * S + (kt + 1) * KT],
                            rhs=wq_sb[k][:, 2 * DPC:3 * DPC],
                            start=(k == 0), stop=(k == HKT - 1),
                        )
                    nc.vector.tensor_add(
                        vaug[b][:, kt // 2, kt % 2, VOFF:VOFF + DPC],
                        v_ps[:], vbias_sb[:],
                    )
                return run

            def qk_group(fg, half):
                tiles = [
                    psp.tile([128, 1024], F32, name=f"qp{fg}{half}a", tag="s"),
                    psp.tile([128, 512], F32, name=f"qp{fg}{half}c", tag="o"),
                    psp.tile([128, 512], F32, name=f"qp{fg}{half}d", tag="y"),
                ]

                def tc_slice(t):
                    if t < 2:
                        return tiles[0][:, t * 512:(t + 1) * 512]
                    return tiles[t - 1][:]

                for k in range(HKT):
                    for t in range(4):
                        nc.tensor.matmul(
                            tc_slice(t),
                            lhsT=wq_sb[k][:, fg * DPC:(fg + 1) * DPC],
                            rhs=xT_sb[k][:, half * S + t * 512:half * S + (t + 1) * 512],
                            start=(k == 0),
                            stop=(k == HKT - 1),
                        )
                for t in range(4):
                    nc.vector.tensor_scalar_add(
                        qkvT_sb[fg][:, half * S + t * 512:half * S + (t + 1) * 512],
                        tc_slice(t), bias_sb[:, fg:fg + 1],
                    )

            qk_group(0, 0)
            qk_group(1, 0)

            qT_sb, kT_sb = qkvT_sb[0], qkvT_sb[1]

            def va_lhsT(b, h, pi):
                # per ko block: head 0 [onesA | vA] cols 15..79; head 1
                # [vB | onesB] cols 80..144.  Returns [128, 2, 65] for the
                # DoubleRow pair pi.
                c0 = 15 if h == 0 else 80
                return vaug[b][:, pi, :, c0:c0 + HD + 1]

            def qk_chunk_step(fg, half, t):
                def run():
                    tg = ("y", "y")[t % 2]
                    ps = psp.tile([128, 512], F32, name=f"qkc{fg}{half}{t}", tag=tg)
                    for k in range(HKT):
                        nc.tensor.matmul(
                            ps[:],
                            lhsT=wq_sb[k][:, fg * DPC:(fg + 1) * DPC],
                            rhs=xT_sb[k][:, half * S + t * 512:half * S + (t + 1) * 512],
                            start=(k == 0),
                            stop=(k == HKT - 1),
                        )
                    nc.vector.tensor_scalar_add(
                        qkvT_sb[fg][:, half * S + t * 512:half * S + (t + 1) * 512],
                        ps[:], bias_sb[:, fg:fg + 1],
                    )
                return run

            # ---- attention + pipelined out-projection epilogue ----
            # pending: filler steps (v tiles, b1 projections, epilogues) run
            # one-or-two per k-tile inside the attention loops.
            pending = [v_tile_step(0, kt) for kt in range(NKT)]
            extra = [v_tile_step(1, kt) for kt in range(NKT)]
            extra += [qk_chunk_step(fg, 1, t) for fg in (0, 1) for t in range(4)]

            def make_epilogue(b, qc, oT):
                q0 = b * S + qc * QC
                state = {}

                def ct_step():
                    ct = [psp.tile([128, 4], F32, name=f"ct{b}{qc}{h}", tag="y")
                          for h in range(HPC)]
                    rT = smallp.tile([128, 2 * 4], F32, name=f"rT{b}{qc}", tag="rT")
                    for h in range(HPC):
                        for tt in range(4):
                            nc.tensor.matmul(
                                ct[h][:, tt:tt + 1],
                                lhsT=oT[h][:, tt * KT:(tt + 1) * KT],
                                rhs=wo_sb[h][:, H:H + 1],
                                start=True, stop=True,
                            )
                        nc.vector.reciprocal(rT[:, h * 4:h * 4 + 4], ct[h][:])
                    state["rT"] = rT

                steps = [ct_step]

                def y_step(tt, ec):
                    def run():
                        rT = state["rT"]
                        y_ps = [
                            psp.tile([128, 512], F32, name=f"y{b}{qc}{tt}{ec}{h}",
                                     tag="y")
                            for h in range(HPC)
                        ]
                        for h in range(HPC):
                            nc.tensor.matmul(
                                y_ps[h][:],
                                lhsT=oT[h][:, tt * KT:(tt + 1) * KT],
                                rhs=wo_sb[h][:, ec * 512:(ec + 1) * 512],
                                start=True, stop=True,
                            )
                        y_sb = ysbp.tile([128, 512], BF16, name=f"ys{b}{qc}{tt}{ec}",
                                         tag="ysb")
                        nc.vector.tensor_scalar_mul(
                            y_sb[:], y_ps[0][:], rT[:, 0 * 4 + tt:0 * 4 + tt + 1])
                        nc.vector.scalar_tensor_tensor(
                            y_sb[:], y_ps[1][:], rT[:, 1 * 4 + tt:1 * 4 + tt + 1],
                            y_sb[:], op0=MULT, op1=ADD,
                        )
                        nc.sync.dma_start(
                            out[q0 + tt * KT:q0 + (tt + 1) * KT,
                                ec * 512:(ec + 1) * 512],
                            y_sb[:],
                        )
                    return run

                for tt in range(4):
                    for ec in range(2):
                        steps.append(y_step(tt, ec))
                return steps

            NPAIR = NKT // 2

            def pv_pair(b, o_ps, p_pairs, pi):
                for h in range(HPC):
                    nc.tensor.matmul(
                        o_ps[h][:],
                        lhsT=va_lhsT(b, h, pi),
                        rhs=p_pairs[pi][:, :, h * QC:(h + 1) * QC],
                        start=(pi == 0), stop=(pi == NPAIR - 1),
                        perf_mode=mybir.MatmulPerfMode.DoubleRow,
                    )

            for b in range(B):
                for qc in range(NQC):
                    q0 = b * S + qc * QC
                    o_ps = [psp.tile([HD + 1, QC], F32, name=f"o{b}{qc}{h}", tag="o")
                            for h in range(HPC)]
                    p_pairs = []
                    for kt in range(NKT):
                        s_ps = psp.tile([128, HPC * QC], F32, tag="s")
                        for h in range(HPC):
                            nc.tensor.matmul(
                                s_ps[:, h * QC:(h + 1) * QC],
                                lhsT=kT_sb[h * HD:(h + 1) * HD,
                                           b * S + kt * KT:b * S + (kt + 1) * KT],
                                rhs=qT_sb[h * HD:(h + 1) * HD, q0:q0 + QC],
                                start=True, stop=True,
                            )
                        if kt % 2 == 0:
                            p_pairs.append(
                                pp.tile([128, 2, HPC * QC], FP8, tag="p",
                                        name=f"pp{b}{qc}{kt}"))
                        nc.scalar.activation(
                            p_pairs[kt // 2][:, kt % 2, :], s_ps[:], EXP,
                            scale=float(EXPSCALE))
                        if pending:
                            pending.pop(0)()
                        if pending and len(pending) > NKT - 1 - kt:
                            pending.pop(0)()
                        if kt % 2 == 1 and kt >= 3:
                            pv_pair(b, o_ps, p_pairs, (kt - 3) // 2)
                    pv_pair(b, o_ps, p_pairs, NPAIR - 1)
                    oT = [oTp.tile([HD + 1, QC], BF16, name=f"oT{b}{qc}{h}", tag="oT")
                          for h in range(HPC)]
                    for h in range(HPC):
                        nc.vector.tensor_copy(oT[h][:], o_ps[h][:])
                    while pending:
                        pending.pop(0)()
                    epi = make_epilogue(b, qc, oT)
                    take = min(len(extra), NKT - len(epi))
                    pending = extra[:take] + epi
                    del extra[:take]
            while pending:
                pending.pop(0)()
    nc.compile()
    return nc


def _get_nc():
    if "nc" not in _CACHED:
        _CACHED["nc"] = _build_nc()
    return _CACHED["nc"]


def _host_prep(x, qkv_w, qkv_b, out_w):
    x = np.asarray(x, dtype=np.float32)
    qkv_w = np.asarray(qkv_w, dtype=np.float32)
    qkv_b = np.asarray(qkv_b, dtype=np.float32)
    out_w = np.asarray(out_w, dtype=np.float32)

    xT = np.ascontiguousarray(x.reshape(T, H).T).astype(BF16_NP)
    in_maps = []
    for c in range(NCORES):
        wq = qkv_w[128 * c:128 * c + 128]
        wk = qkv_w[H + 128 * c:H + 128 * c + 128]
        wv = qkv_w[2 * H + 128 * c:2 * H + 128 * c + 128]
        wqkvT = np.ascontiguousarray(np.concatenate([wq, wk, wv], 0).T).astype(BF16_NP)
        bq = np.stack(
            [qkv_b[fg * H + 128 * c:fg * H + 128 * c + 128] for fg in range(3)],
            axis=1,
        ).astype(np.float32)
        woTa = np.zeros((HPC, HD + 1, H + 1), np.float32)
        for h in range(HPC):
            g = HPC * c + h
            w = out_w[:, g * HD:(g + 1) * HD].T  # [64, 1024]
            if h == 0:
                # head A: colsum is row 0 of o_aug (ones col precedes v)
                woTa[h, 1:HD + 1, 0:H] = w
                woTa[h, 0, H] = 1.0
            else:
                # head B: colsum is row 64 (ones col follows v)
                woTa[h, 0:HD, 0:H] = w
                woTa[h, HD, H] = 1.0
        vb = np.broadcast_to(
            qkv_b[2 * H + 128 * c:2 * H + 128 * c + 128][None, :], (128, DPC)
        ).astype(np.float32)
        in_maps.append({
            "xT": xT,
            "wqkvT": wqkvT,
            "bqkv": np.ascontiguousarray(bq),
            "woTa": woTa.astype(BF16_NP),
            "vbias": np.ascontiguousarray(vb),
        })
    return in_maps


def _run(in_maps, trace=False):
    # The image's antenv lacks axon_hooks; register the NTFF profile hook so
    # run_bass_kernel_spmd(trace=True) can report exec_time_ns.
    if trace and "antenv.axon_hooks" not in sys.modules:
        try:
            import trn_agent_boot.trn_boot as _tb
            _hook = _tb._ntff_profile_via_ctypes("/opt/axon/libaxon_pjrt.so")
            _m = types.ModuleType("antenv.axon_hooks")
            _m.get_axon_ntff_profile_hook = lambda: _hook
            sys.modules["antenv.axon_hooks"] = _m
        except Exception:
            trace = False
    from concourse.bass_utils import run_bass_kernel_spmd

    nc = _get_nc()
    res = run_bass_kernel_spmd(nc, in_maps, core_ids=list(range(NCORES)), trace=trace)
    return res


def kernel(x, qkv_w, qkv_b, out_w, out_b):
    in_maps = _host_prep(x, qkv_w, qkv_b, out_w)
    res = _run(in_maps, trace=False)
    total = np.zeros((T, H), np.float32)
    for c in range(NCORES):
        total += np.asarray(res.results[c]["out"], dtype=np.float32)
    total += np.asarray(out_b, dtype=np.float32)[None, :]
    return total.reshape(B, S, H)

